# revision 7
# baseline (speedup 1.0000x reference)
"""Trainium2 Bass kernel for nn_EmbeddingLayer (ViT patch-embedding block).

Pipeline (per token): patchify -> LayerNorm(147) -> int8 absmax fake-quant ->
BitLinear matmul (ternary weights) -> LayerNorm(1024) -> + sincos posemb.

Sharding: data-parallel over batch, 8 images per core across 8 NeuronCores.

Device strategy per core (8192 tokens, 64 tiles of 128 tokens):
  - LN1 stats via bn_stats/bn_aggr, absmax of centered tile, quantize with
    round-to-nearest-even via the +/-1.5*2^23 magic constant, output bf16
    integers (exact).
  - The matmul runs in bf16 with exact integer arithmetic (quantized acts in
    [-127,127], ternary weights in {-1,0,1}); scales are factored out:
    z = alpha * S + b, with the bias folded into the matmul as an extra
    contraction row whose activation coefficient is 1/alpha, and the LN2 mean
    folded in as an extra output column (row-sums of the extended weights).
  - LN2 variance via one ACT Square pass with accumulation; final affine
    (z - mu) * rstd is a per-token scale/bias applied straight out of PSUM,
    split across the ACT and DVE engines.
  - The positional embedding is added *inside PSUM* by an extra matmul with a
    diagonal stationary matrix carrying 1/A per token (A = alpha * rstd2), so
    no full-size elementwise add is needed.
"""

import os

import numpy as np
import ml_dtypes

B, C, H, W_IMG = 64, 3, 224, 224
P = 7
GH, GW = H // P, W_IMG // P        # 32 x 32 = 1024 patches
NPATCH = GH * GW                   # 1024
PD = C * P * P                     # 147
D = 1024
EPS = 1e-5
NCORES = 8
B_CORE = B // NCORES               # 8 images per core
TOK = B_CORE * NPATCH              # 8192 tokens per core
TILE_T = 128                       # tokens per tile
NTILES = TOK // TILE_T             # 64
GRP = 8                            # tiles per small-vector batching group
KEXT = PD + 1                      # 148: contraction with bias row
K0, K1 = 128, KEXT - 128           # K chunks 128 + 20
NOUT = D + 1                       # 1025: output cols + row-sum column
MAGIC = 12582912.0                 # 1.5 * 2**23, fp32 RNE rounding trick
AS_ACT = 768                       # columns of final pass on ACT (rest on DVE)

_cached = {}


def _posemb_sincos_2d(h, w, dim, temperature=10000.0):
    y, x = np.meshgrid(np.arange(h, dtype=np.float32),
                       np.arange(w, dtype=np.float32), indexing="ij")
    omega = np.arange(dim // 4, dtype=np.float32) / np.float32(dim // 4 - 1)
    omega = (1.0 / (temperature ** omega)).astype(np.float32)
    yy = y.reshape(-1, 1) * omega
    xx = x.reshape(-1, 1) * omega
    return np.concatenate(
        [np.sin(xx), np.cos(xx), np.sin(yy), np.cos(yy)], axis=1
    ).astype(np.float32)


def _reference_numpy(x, ln1_g, ln1_b, W_proj, b_proj, ln2_g, ln2_b):
    """General-path fallback; exact port of the reference in numpy fp32."""
    x = x.astype(np.float32)
    p = x.reshape(B, C, GH, P, GW, P)
    p = p.transpose(0, 2, 4, 3, 5, 1).reshape(B, NPATCH, PD)

    def layernorm(v, g, b):
        mu = v.mean(-1, keepdims=True, dtype=np.float32)
        var = np.square(v - mu).mean(-1, keepdims=True, dtype=np.float32)
        return (v - mu) / np.sqrt(var + EPS) * g + b

    p = layernorm(p, ln1_g, ln1_b)
    s_x = 127.0 / np.clip(np.max(np.abs(p), -1, keepdims=True), 1e-5, None)
    xq = np.clip(np.round(p * s_x), -128, 127) / s_x
    s_w = 1.0 / np.clip(np.mean(np.abs(W_proj)), 1e-5, None)
    Wq = np.clip(np.round(W_proj * s_w), -1, 1) / s_w
    p = np.einsum("bnp,dp->bnd", xq, Wq, dtype=np.float32) + b_proj
    p = layernorm(p, ln2_g, ln2_b)
    pe = _posemb_sincos_2d(GH, GW, D)
    return (p + pe).astype(np.float32)


def _build_bass():
    from contextlib import ExitStack

    import concourse.bacc as bacc
    import concourse.bass as bass
    import concourse.tile as tile
    from concourse import mybir

    f32 = mybir.dt.float32
    bf16 = mybir.dt.bfloat16
    Alu = mybir.AluOpType
    Act = mybir.ActivationFunctionType

    nc = bacc.Bacc(trn_type="TRN2", target_bir_lowering=False, debug=False,
                   num_devices=NCORES)

    xp = nc.dram_tensor("xp", [TOK, PD], f32, kind="ExternalInput")
    wk0 = nc.dram_tensor("wk0", [K0, NOUT], bf16, kind="ExternalInput")
    wk1 = nc.dram_tensor("wk1", [K1, NOUT], bf16, kind="ExternalInput")
    pe_d = nc.dram_tensor("pe", [NPATCH, D], bf16, kind="ExternalInput")
    ident_d = nc.dram_tensor("ident", [128, 128], bf16, kind="ExternalInput")
    consts_d = nc.dram_tensor("consts", [1], f32, kind="ExternalInput")
    out_d = nc.dram_tensor("out", [TOK, D], f32, kind="ExternalOutput")

    with tile.TileContext(nc) as tc, ExitStack() as ctx:
        singles = ctx.enter_context(tc.tile_pool(name="singles", bufs=1))
        p_pool = ctx.enter_context(tc.tile_pool(name="p", bufs=10))
        c_pool = ctx.enter_context(tc.tile_pool(name="c", bufs=10))
        st_pool = ctx.enter_context(tc.tile_pool(name="st", bufs=10))
        grp_pool = ctx.enter_context(tc.tile_pool(name="grp", bufs=2))
        q_pool = ctx.enter_context(tc.tile_pool(name="q", bufs=3))
        qx_pool = ctx.enter_context(tc.tile_pool(name="qx", bufs=3))
        qxt_pool = ctx.enter_context(tc.tile_pool(name="qxt", bufs=3))
        diag_pool = ctx.enter_context(tc.tile_pool(name="diag", bufs=2))
        sv_pool = ctx.enter_context(tc.tile_pool(name="sv", bufs=4))
        scr_pool = ctx.enter_context(tc.tile_pool(name="scr", bufs=2))
        out_pool = ctx.enter_context(tc.tile_pool(name="outp", bufs=3))
        ps_pool = ctx.enter_context(
            tc.tile_pool(name="ps", bufs=2, space="PSUM"))
        pt_pool = ctx.enter_context(
            tc.tile_pool(name="pt", bufs=2, space="PSUM"))

        # --- one-time loads -------------------------------------------------
        wk0_sb = singles.tile([K0, NOUT], bf16)
        nc.sync.dma_start(wk0_sb[:], wk0[:, :])
        wk1_sb = singles.tile([K1, NOUT], bf16)
        nc.sync.dma_start(wk1_sb[:], wk1[:, :])
        ident = singles.tile([128, 128], bf16)
        nc.sync.dma_start(ident[:], ident_d[:, :])
        pe_sb = singles.tile([128, B_CORE, D], bf16)
        nc.sync.dma_start(
            pe_sb[:], pe_d.ap().rearrange("(g p) d -> p g d", p=128))
        k2_sb = singles.tile([128, 1], f32)
        nc.sync.dma_start(
            k2_sb[:],
            bass.AP(tensor=consts_d, offset=0, ap=[[0, 128], [1, 1]]))

        for g in range(NTILES // GRP):
            mv_g = grp_pool.tile([128, GRP, 2], f32, tag="mv")
            m_g = grp_pool.tile([128, GRP, 1], f32, tag="m")
            c_tiles = []
            for j in range(GRP):
                t = g * GRP + j
                p_t = p_pool.tile([128, PD], f32)
                nc.sync.dma_start(p_t[:], xp[t * TILE_T:(t + 1) * TILE_T, :])
                st6 = st_pool.tile([128, 6], f32)
                nc.vector.bn_stats(out=st6[:], in_=p_t[:])
                nc.vector.bn_aggr(out=mv_g[:, j, :], in_=st6[:])
                c_t = c_pool.tile([128, PD], f32)
                nc.vector.tensor_scalar_sub(c_t[:], p_t[:], mv_g[:, j, 0:1])
                nc.vector.tensor_reduce(
                    out=m_g[:, j, :], in_=c_t[:], axis=mybir.AxisListType.X,
                    op=Alu.max, apply_absolute_value=True)
                c_tiles.append(c_t)

            # --- batched per-group small-vector chain [128, GRP] ------------
            mc = grp_pool.tile([128, GRP, 1], f32, tag="mc")
            nc.vector.tensor_scalar_max(mc[:], m_g[:], 1e-5)
            sr = grp_pool.tile([128, GRP, 1], f32, tag="sr")
            nc.vector.reciprocal(sr[:], mc[:])
            s127 = grp_pool.tile([128, GRP, 1], f32, tag="s127")
            nc.vector.tensor_scalar_mul(s127[:], sr[:], 127.0)
            v1e = grp_pool.tile([128, GRP, 1], f32, tag="v1e")
            nc.vector.tensor_scalar_add(v1e[:], mv_g[:, :, 1:2], EPS)
            sv = grp_pool.tile([128, GRP, 1], f32, tag="sv")
            nc.scalar.activation(sv[:], v1e[:], Act.Sqrt)
            rr = grp_pool.tile([128, GRP, 1], f32, tag="rr")
            nc.vector.tensor_tensor(
                out=rr[:], in0=sv[:], in1=sr[:], op=Alu.mult)
            nc.vector.tensor_scalar_min(rr[:], rr[:], 1e5)
            inva = grp_pool.tile([128, GRP, 1], f32, tag="inva")
            nc.vector.tensor_scalar_mul(inva[:], rr[:], k2_sb[:, 0:1])
            # epsd = inva^2 * EPS
            epsd = grp_pool.tile([128, GRP, 1], f32, tag="epsd")
            iva2 = grp_pool.tile([128, GRP, 1], f32, tag="iva2")
            nc.vector.tensor_tensor(
                out=iva2[:], in0=inva[:], in1=inva[:], op=Alu.mult)
            nc.vector.tensor_scalar_mul(epsd[:], iva2[:], EPS)

            for j in range(GRP):
                t = g * GRP + j
                img = t // (NPATCH // TILE_T)
                pos = t % (NPATCH // TILE_T)
                c_t = c_tiles[j]

                # --- quantize to integer bf16 ------------------------------
                cq = q_pool.tile([128, PD], f32)
                nc.vector.tensor_scalar(
                    out=cq[:], in0=c_t[:], scalar1=s127[:, j, :],
                    scalar2=MAGIC, op0=Alu.mult, op1=Alu.add)
                qx = qx_pool.tile([128, KEXT], bf16)
                nc.vector.tensor_scalar_sub(qx[:, 0:PD], cq[:], MAGIC)
                nc.vector.tensor_copy(qx[:, PD:KEXT], inva[:, j, :])

                # --- transpose the stationary operand via PE ----------------
                pt_ps = pt_pool.tile([128, 256], bf16)
                nc.tensor.transpose(pt_ps[:, 0:128], qx[:, 0:K0], ident[:])
                nc.tensor.transpose(
                    pt_ps[0:K1, 128:256], qx[:, K0:KEXT], ident[:])
                qxt_a = qxt_pool.tile([128, 128], bf16, tag="qxta")
                nc.vector.tensor_copy(qxt_a[:], pt_ps[:, 0:128])
                qxt_b = qxt_pool.tile([K1, 128], bf16, tag="qxtb")
                nc.scalar.copy(qxt_b[:], pt_ps[0:K1, 128:256])

                # --- main matmuls ------------------------------------------
                s_ps = ps_pool.tile([128, 1032], f32)
                nc.tensor.matmul(s_ps[:, 0:512], qxt_a[:], wk0_sb[:, 0:512],
                                 start=True, stop=False)
                nc.tensor.matmul(s_ps[:, 0:512], qxt_b[:], wk1_sb[:, 0:512],
                                 start=False, stop=False)
                nc.tensor.matmul(s_ps[:, 512:1024], qxt_a[:],
                                 wk0_sb[:, 512:1024], start=True, stop=False)
                nc.tensor.matmul(s_ps[:, 512:1024], qxt_b[:],
                                 wk1_sb[:, 512:1024], start=False, stop=False)
                nc.tensor.matmul(s_ps[:, 1024:1025], qxt_a[:],
                                 wk0_sb[:, 1024:1025], start=True, stop=False)
                nc.tensor.matmul(s_ps[:, 1024:1025], qxt_b[:],
                                 wk1_sb[:, 1024:1025], start=False, stop=True)

                # --- LN2 stats ---------------------------------------------
                ssq = sv_pool.tile([128, 1], f32, tag="ssq")
                sq_scr = scr_pool.tile([128, 1024], bf16)
                nc.scalar.activation(sq_scr[:], s_ps[:, 0:1024], Act.Square,
                                     accum_out=ssq[:])
                nm = sv_pool.tile([128, 1], f32, tag="nm")
                nc.vector.tensor_scalar_mul(
                    nm[:], s_ps[:, 1024:1025], -1.0 / D)
                nm2 = sv_pool.tile([128, 1], f32, tag="nm2")
                nc.vector.tensor_tensor(
                    out=nm2[:], in0=nm[:], in1=nm[:], op=Alu.mult)
                # var = ssq/D - nm^2 ; den = var + epsd
                var = sv_pool.tile([128, 1], f32, tag="var")
                nc.vector.tensor_scalar(
                    out=var[:], in0=ssq[:], scalar1=1.0 / D, scalar2=nm2[:],
                    op0=Alu.mult, op1=Alu.subtract)
                den = sv_pool.tile([128, 1], f32, tag="den")
                nc.vector.tensor_tensor(
                    out=den[:], in0=var[:], in1=epsd[:, j, :], op=Alu.add)
                sqd = sv_pool.tile([128, 1], f32, tag="sqd")
                nc.scalar.activation(sqd[:], den[:], Act.Sqrt)
                a_sc = sv_pool.tile([128, 1], f32, tag="asc")
                nc.vector.reciprocal(a_sc[:], sqd[:])
                c_sc = sv_pool.tile([128, 1], f32, tag="csc")
                nc.vector.tensor_tensor(
                    out=c_sc[:], in0=a_sc[:], in1=nm[:], op=Alu.mult)

                # --- add posemb via diagonal matmul ------------------------
                diag = diag_pool.tile([128, 128], bf16)
                nc.vector.tensor_scalar_mul(diag[:], ident[:], sqd[:])
                nc.tensor.matmul(s_ps[:, 0:512], diag[:],
                                 pe_sb[:, pos, 0:512], start=False, stop=True)
                nc.tensor.matmul(s_ps[:, 512:1024], diag[:],
                                 pe_sb[:, pos, 512:1024],
                                 start=False, stop=True)

                # --- final affine out = S' * A + C, split ACT/DVE ----------
                o_t = out_pool.tile([128, D], f32)
                nc.scalar.activation(o_t[:, 0:AS_ACT], s_ps[:, 0:AS_ACT],
                                     Act.Identity, scale=a_sc[:], bias=c_sc[:])
                nc.vector.tensor_scalar(
                    out=o_t[:, AS_ACT:D], in0=s_ps[:, AS_ACT:D],
                    scalar1=a_sc[:], scalar2=c_sc[:],
                    op0=Alu.mult, op1=Alu.add)
                nc.sync.dma_start(
                    out_d[t * TILE_T:(t + 1) * TILE_T, :], o_t[:])

    nc.compile()
    return nc


def _host_prep(x, b_proj, W_proj, ln2_b):
    bf16 = ml_dtypes.bfloat16
    xp = x.reshape(B, C, GH, P, GW, P).transpose(0, 2, 4, 3, 5, 1)
    xp = np.ascontiguousarray(xp.reshape(B, NPATCH, PD), dtype=np.float32)

    inv_sw = np.float32(max(np.float32(np.mean(np.abs(W_proj))),
                            np.float32(1e-5)))
    s_w = np.float32(1.0) / inv_sw
    wq = np.clip(np.round(W_proj.astype(np.float32) * s_w), -1, 1)
    w_ext = np.concatenate([wq.T, b_proj[None, :].astype(np.float32)], axis=0)
    u = w_ext.sum(axis=1, dtype=np.float32)
    w_full = np.concatenate([w_ext, u[:, None]], axis=1).astype(bf16)

    pe = _posemb_sincos_2d(GH, GW, D) + ln2_b.astype(np.float32)
    k2 = np.asarray([127.0 / inv_sw], dtype=np.float32)
    ident = np.eye(128, dtype=bf16)
    return (xp, w_full[:K0].copy(), w_full[K0:].copy(),
            pe.astype(bf16), ident, k2)


def kernel(x, ln1_g, ln1_b, W_proj, b_proj, ln2_g, ln2_b):
    x = np.asarray(x, dtype=np.float32)
    ln1_g = np.asarray(ln1_g, np.float32)
    ln1_b = np.asarray(ln1_b, np.float32)
    W_proj = np.asarray(W_proj, np.float32)
    b_proj = np.asarray(b_proj, np.float32)
    ln2_g = np.asarray(ln2_g, np.float32)
    ln2_b = np.asarray(ln2_b, np.float32)

    # The device kernel exploits ln1_g == 1, ln1_b == 0, ln2_g == 1 (the
    # values produced by setup_inputs); fall back to a full numpy port of the
    # reference for any other parameters.
    if not (np.all(ln1_g == 1.0) and np.all(ln1_b == 0.0)
            and np.all(ln2_g == 1.0)):
        return _reference_numpy(x, ln1_g, ln1_b, W_proj, b_proj, ln2_g, ln2_b)

    from concourse.bass_utils import run_bass_kernel_spmd

    xp, wk0, wk1, pe, ident, k2 = _host_prep(x, b_proj, W_proj, ln2_b)

    if "nc" not in _cached:
        _cached["nc"] = _build_bass()
    nc = _cached["nc"]

    in_maps = []
    for c in range(NCORES):
        shard = np.ascontiguousarray(
            xp[c * B_CORE:(c + 1) * B_CORE].reshape(TOK, PD))
        in_maps.append({"xp": shard, "wk0": wk0, "wk1": wk1, "pe": pe,
                        "ident": ident, "consts": k2})

    trace = bool(int(os.environ.get("BASSK_TRACE", "0")))
    res = run_bass_kernel_spmd(nc, in_maps, core_ids=list(range(NCORES)),
                               trace=trace)
    _cached["last_result"] = res

    out = np.concatenate([r["out"].reshape(B_CORE, NPATCH, D)
                          for r in res.results], axis=0)
    return out.astype(np.float32)
